# revision 9
# baseline (speedup 1.0000x reference)
"""Trainium2 Bass kernel for the DVS-SNN problem.

Model (per reference):
    for t in 0..T-1:
        i1 = x_t @ w1.T + spk @ w_rec.T
        v1 = v1 + i1 - LEAK ; spk = (v1 >= 1) ; v1 -= spk
        v2 = max(v2 + spk @ w2.T - OUTPUT_LEAK, 0) ; out_sum += v2
    return out_sum / T

Strategy: data-parallel over batch (64 = 8 cores x 8). Per core:
  Phase A (time-parallel): XprojT[h, t*8+b] = (x_t @ w1.T).T  computed as
      one big matmul with x pre-transposed on host to xt[c, t*8+b].
  Phase B (sequential scan over T): state kept transposed
      [H on partitions (4 chunks of 128 in the free dim), B on free]:
      tiles are [128, 32] with free index = 8*h_chunk + b.
      rec.T = w_rec @ spk.T via 16 matmuls (stationary = w_rec.T chunks).
  v1 dynamics are scaled by SCALE (power of 2, numerically transparent)
  so fp16 w_rec stays out of subnormal range.

Modes (env SNN_MODE): "fp32" (exact) | "fp16" (w_rec/w2/spk fp16,
x/w1 split hi+lo fp16 3-pass matmul; ~exact, much faster).
"""

import os
import numpy as np

B, T, C, H, O = 64, 256, 2048, 512, 11
THRESHOLD = 1.0
LEAK = 0.003
OUTPUT_LEAK = LEAK * 0.5

NCORES = 8
BL = B // NCORES          # batch per core = 8
BT = T * BL               # 2048 moving columns per core
P = 128
KC = C // P               # 16 contraction chunks for phase A
KH = H // P               # 4 H chunks
NTILE = 512               # phase A moving tile (64 timesteps x 8 batch)
NT = BT // NTILE          # 4 BT tiles

MODE = os.environ.get("SNN_MODE", "fp16")
DEBUG_OUT = os.environ.get("SNN_DEBUG_OUT", "0") == "1"

# fp16 mode: scale v1 dynamics so fp16(SCALE*w_rec) avoids subnormals.
SCALE = 64.0 if MODE == "fp16" else 1.0
LO_SHIFT = 2048.0  # 2^11: scaling for the w1 low-part in fp16 3-pass


def build_nc(t_steps=T, mode=MODE, debug_out=DEBUG_OUT):
    """Build the Bass program (same program for all 8 cores)."""
    from contextlib import ExitStack

    import concourse.bass as bass
    import concourse.tile as tile
    from concourse import bacc, mybir

    f32 = mybir.dt.float32
    f16 = mybir.dt.float16
    wdt = f16 if mode == "fp16" else f32
    alu = mybir.AluOpType
    ACT = mybir.ActivationFunctionType

    nc = bacc.Bacc("TRN2", target_bir_lowering=False, debug=False,
                   num_devices=NCORES)

    # ---- DRAM I/O ----
    if mode == "fp16":
        xh_d = nc.dram_tensor("xh", [C, BT], f16, kind="ExternalInput")
        xl_d = nc.dram_tensor("xl", [C, BT], f16, kind="ExternalInput")
        xs_d = nc.dram_tensor("xs", [C, BT], f16, kind="ExternalInput")
        w1h_d = nc.dram_tensor("w1h", [C, H], f16, kind="ExternalInput")
        w1l_d = nc.dram_tensor("w1l", [C, H], f16, kind="ExternalInput")
    else:
        xt_d = nc.dram_tensor("xt", [C, BT], f32, kind="ExternalInput")
        w1t_d = nc.dram_tensor("w1t", [C, H], f32, kind="ExternalInput")
    wrt_d = nc.dram_tensor("wrt", [H, H], wdt, kind="ExternalInput")
    w2t_d = nc.dram_tensor("w2t", [H, O], wdt, kind="ExternalInput")
    out_d = nc.dram_tensor("out", [O, BL], f32, kind="ExternalOutput")
    if debug_out:
        v1_d = nc.dram_tensor("v1_dbg", [P, KH * BL], f32, kind="ExternalOutput")
        xp_d = nc.dram_tensor("xp_dbg", [P, 32 * t_steps], f32, kind="ExternalOutput")

    with tile.TileContext(nc) as tc, ExitStack() as ctx:
        # ---- persistent tiles (one pool, one slot per tag) ----
        perm = ctx.enter_context(tc.tile_pool(name="perm", bufs=1))
        xproj = perm.tile([P, 32 * t_steps], f32, tag="xproj", name="xproj")
        v1 = perm.tile([P, KH * BL], f32, tag="v1", name="v1")
        v2pre = perm.tile([O, BL], f32, tag="v2pre", name="v2pre")
        osum = perm.tile([O, BL], f32, tag="osum", name="osum")

        w1_sb = []
        if mode == "fp16":
            for k in range(KC):
                w1h_k = perm.tile([P, H], f16, tag=f"w1h_{k}", name=f"w1h_{k}")
                w1l_k = perm.tile([P, H], f16, tag=f"w1l_{k}", name=f"w1l_{k}")
                nc.sync.dma_start(out=w1h_k[:], in_=w1h_d.ap()[k * P:(k + 1) * P, :])
                nc.sync.dma_start(out=w1l_k[:], in_=w1l_d.ap()[k * P:(k + 1) * P, :])
                w1_sb.append((w1h_k, w1l_k))
        else:
            for k in range(KC):
                w1_k = perm.tile([P, H], f32, tag=f"w1_{k}", name=f"w1_{k}")
                nc.sync.dma_start(out=w1_k[:], in_=w1t_d.ap()[k * P:(k + 1) * P, :])
                w1_sb.append(w1_k)

        wr_sb = []
        w2_sb = []
        for k in range(KH):
            wr_k = perm.tile([P, H], wdt, tag=f"wr_{k}", name=f"wr_{k}")
            nc.sync.dma_start(out=wr_k[:], in_=wrt_d.ap()[k * P:(k + 1) * P, :])
            wr_sb.append(wr_k)
            w2_k = perm.tile([P, O], wdt, tag=f"w2_{k}", name=f"w2_{k}")
            nc.sync.dma_start(out=w2_k[:], in_=w2t_d.ap()[k * P:(k + 1) * P, :])
            w2_sb.append(w2_k)

        nc.vector.memset(v1[:], 0.0)
        nc.vector.memset(v2pre[:], 0.0)
        nc.vector.memset(osum[:], 0.0)

        xt_pool = ctx.enter_context(tc.tile_pool(name="xt", bufs=4))
        psA_pool = ctx.enter_context(tc.tile_pool(name="psA", bufs=1, space="PSUM"))
        psB_pool = ctx.enter_context(tc.tile_pool(name="psB", bufs=2, space="PSUM"))
        psV_pool = ctx.enter_context(tc.tile_pool(name="psV", bufs=2, space="PSUM"))
        spk_pool = ctx.enter_context(tc.tile_pool(name="spk", bufs=3))

        # ================= Phase A: XprojT =================
        # xproj[p, 32t + 8j + b] = SCALE * ((x_t @ w1.T)[b, 128j+p] - LEAK)
        ntile = min(NTILE, t_steps * BL)
        n_steps_per_tile = ntile // BL
        nA = (t_steps * BL) // ntile
        for n in range(nA):
            psA = [psA_pool.tile([P, ntile], f32, tag=f"psA{j}", name=f"psA{j}") for j in range(KH)]
            for k in range(KC):
                csl = slice(k * P, (k + 1) * P)
                nsl = slice(n * ntile, (n + 1) * ntile)
                if mode == "fp16":
                    xh_t = xt_pool.tile([P, ntile], f16, tag="xh", name="xh_t")
                    xl_t = xt_pool.tile([P, ntile], f16, tag="xl", name="xl_t")
                    xs_t = xt_pool.tile([P, ntile], f16, tag="xs", name="xs_t")
                    nc.sync.dma_start(out=xh_t[:], in_=xh_d.ap()[csl, nsl])
                    nc.sync.dma_start(out=xl_t[:], in_=xl_d.ap()[csl, nsl])
                    nc.sync.dma_start(out=xs_t[:], in_=xs_d.ap()[csl, nsl])
                    for j in range(KH):
                        st_h = w1_sb[k][0][:][:, j * P:(j + 1) * P]
                        st_l = w1_sb[k][1][:][:, j * P:(j + 1) * P]
                        nc.tensor.matmul(psA[j][:], st_h, xh_t[:],
                                         start=(k == 0), stop=False)
                        nc.tensor.matmul(psA[j][:], st_h, xl_t[:],
                                         start=False, stop=False)
                        nc.tensor.matmul(psA[j][:], st_l, xs_t[:],
                                         start=False, stop=(k == KC - 1))
                else:
                    xt_t = xt_pool.tile([P, ntile], f32, tag="xt", name="xt_t")
                    nc.sync.dma_start(out=xt_t[:], in_=xt_d.ap()[csl, nsl])
                    for j in range(KH):
                        st = w1_sb[k][:][:, j * P:(j + 1) * P]
                        nc.tensor.matmul(psA[j][:], st, xt_t[:],
                                         start=(k == 0), stop=(k == KC - 1))
            # copy PSUM -> xproj (strided dest), fusing scale and leak
            for j in range(KH):
                dest = xproj[:].rearrange("p (t j b) -> p t j b", j=KH, b=BL)[
                    :, n * n_steps_per_tile:(n + 1) * n_steps_per_tile, j, :]
                src = psA[j][:].rearrange("p (t b) -> p t b", b=BL)
                nc.scalar.activation(dest, src, ACT.Copy,
                                     bias=-SCALE * LEAK, scale=SCALE)

        # ================= Phase B: sequential scan =================
        TH_S = SCALE * THRESHOLD
        spk_dt = f16 if mode == "fp16" else f32
        psB_prev = None
        psV_prev = None
        for t in range(t_steps):
            # ---- v1 / spike update for step t ----
            if psB_prev is not None:
                # v1 += rec (split in two halves for latency overlap)
                nc.vector.tensor_add(v1[:, 0:16], v1[:, 0:16], psB_prev[:, 0:16])
            nc.gpsimd.tensor_add(v1[:, 0:16], v1[:, 0:16],
                                 xproj[:, 32 * t:32 * t + 16])
            spk = spk_pool.tile([P, KH * BL], spk_dt, tag="spk", name="spk")
            nc.vector.tensor_scalar(spk[:, 0:16], v1[:, 0:16], TH_S, None,
                                    alu.is_ge)
            if psB_prev is not None:
                nc.vector.tensor_add(v1[:, 16:32], v1[:, 16:32], psB_prev[:, 16:32])
            nc.gpsimd.tensor_add(v1[:, 16:32], v1[:, 16:32],
                                 xproj[:, 32 * t + 16:32 * t + 32])
            nc.vector.tensor_scalar(spk[:, 16:32], v1[:, 16:32], TH_S, None,
                                    alu.is_ge)
            # subtractive reset: v1 = (-TH_S * spk) + v1  (STT is DVE-only)
            nc.vector.scalar_tensor_tensor(v1[:], spk[:], -TH_S, v1[:],
                                           alu.mult, alu.add)

            # ---- v2 path for step t-1 (uses psV_prev) ----
            if psV_prev is not None:
                tmpv = spk_pool.tile([O, BL], f32, tag="tmpv", name="tmpv")
                # tmp = max(v2pre, 0) + i2
                nc.vector.scalar_tensor_tensor(tmpv[:], v2pre[:], 0.0,
                                               psV_prev[:], alu.max, alu.add)
                nc.vector.tensor_scalar(v2pre[:], tmpv[:], -OUTPUT_LEAK, None,
                                        alu.add)
                # osum += max(v2pre, 0)
                nc.vector.scalar_tensor_tensor(osum[:], v2pre[:], 0.0,
                                               osum[:], alu.max, alu.add)

            # ---- PE: rec and i2 for step t ----
            psB = psB_pool.tile([P, KH * BL], f32, tag="psB", name="psB")
            for j in range(KH):
                for k in range(KH):
                    nc.tensor.matmul(
                        psB[:, BL * j:BL * (j + 1)],
                        wr_sb[k][:][:, j * P:(j + 1) * P],
                        spk[:, BL * k:BL * (k + 1)],
                        start=(k == 0), stop=(k == KH - 1))
            psV = psV_pool.tile([O, BL], f32, tag="psV", name="psV")
            for k in range(KH):
                nc.tensor.matmul(psV[:], w2_sb[k][:],
                                 spk[:, BL * k:BL * (k + 1)],
                                 start=(k == 0), stop=(k == KH - 1))
            psB_prev, psV_prev = psB, psV

        # ---- final flush: v2 path for the last step ----
        tmpv = spk_pool.tile([O, BL], f32, tag="tmpv", name="tmpv")
        nc.vector.scalar_tensor_tensor(tmpv[:], v2pre[:], 0.0, psV_prev[:],
                                       alu.max, alu.add)
        nc.vector.tensor_scalar(v2pre[:], tmpv[:], -OUTPUT_LEAK, None, alu.add)
        nc.vector.scalar_tensor_tensor(osum[:], v2pre[:], 0.0, osum[:],
                                       alu.max, alu.add)

        nc.scalar.activation(osum[:], osum[:], ACT.Copy, bias=0.0,
                             scale=1.0 / float(T))
        nc.sync.dma_start(out=out_d.ap()[:, :], in_=osum[:])
        if debug_out:
            nc.sync.dma_start(out=v1_d.ap()[:, :], in_=v1[:])
            nc.sync.dma_start(out=xp_d.ap()[:, :], in_=xproj[:])

    nc.compile()
    return nc


def prep_inputs(x, w1, w_rec, w2, mode=MODE):
    """Build per-core input maps. Host-side transposes/splits (not timed)."""
    x = np.ascontiguousarray(x, dtype=np.float32)
    w1 = np.ascontiguousarray(w1, dtype=np.float32)
    w_rec = np.ascontiguousarray(w_rec, dtype=np.float32)
    w2 = np.ascontiguousarray(w2, dtype=np.float32)

    in_maps = []
    if mode == "fp16":
        w1h = w1.T.astype(np.float16)                              # [C, H]
        w1l = ((w1.T - w1h.astype(np.float32)) * LO_SHIFT).astype(np.float16)
        wrt = (w_rec.T * SCALE).astype(np.float16)                 # [H, H]
        w2t = w2.T.astype(np.float16)                              # [H, O]
        for c in range(NCORES):
            xc = x[c * BL:(c + 1) * BL]                            # [BL, T, C]
            xt = np.ascontiguousarray(xc.transpose(2, 1, 0).reshape(C, BT))
            xh = xt.astype(np.float16)
            xl = (xt - xh.astype(np.float32)).astype(np.float16)
            xs = (xh.astype(np.float32) / LO_SHIFT).astype(np.float16)
            in_maps.append({"xh": xh, "xl": xl, "xs": xs,
                            "w1h": w1h, "w1l": w1l, "wrt": wrt, "w2t": w2t})
    else:
        w1t = np.ascontiguousarray(w1.T)
        wrt = np.ascontiguousarray(w_rec.T) * np.float32(SCALE)
        w2t = np.ascontiguousarray(w2.T)
        for c in range(NCORES):
            xc = x[c * BL:(c + 1) * BL]
            xt = np.ascontiguousarray(xc.transpose(2, 1, 0).reshape(C, BT))
            in_maps.append({"xt": xt, "w1t": w1t, "wrt": wrt, "w2t": w2t})
    return in_maps


_LAST = {"exec_time_ns": None, "results": None}


def kernel(x, w1, w_rec, w2):
    from concourse.bass_utils import run_bass_kernel_spmd

    nc = build_nc()
    in_maps = prep_inputs(x, w1, w_rec, w2)
    trace = os.environ.get("SNN_TRACE", "0") == "1"
    if trace:
        try:
            import antenv
            if "/opt/trn_rl_repo/antenv" not in antenv.__path__:
                antenv.__path__.append("/opt/trn_rl_repo/antenv")
            import antenv.axon_hooks  # noqa: F401
        except Exception:
            trace = False
    res = run_bass_kernel_spmd(nc, in_maps, list(range(NCORES)), trace=trace)
    _LAST["exec_time_ns"] = res.exec_time_ns
    _LAST["results"] = res
    out = np.empty((B, O), dtype=np.float32)
    for c in range(NCORES):
        out[c * BL:(c + 1) * BL, :] = res.results[c]["out"].T
    return out


# revision 15
# speedup vs baseline: 1.1236x; 1.1236x over previous
"""Trainium2 Bass kernel for the DVS-SNN problem.

Model (per reference):
    for t in 0..T-1:
        i1 = x_t @ w1.T + spk @ w_rec.T
        v1 = v1 + i1 - LEAK ; spk = (v1 >= 1) ; v1 -= spk
        v2 = max(v2 + spk @ w2.T - OUTPUT_LEAK, 0) ; out_sum += v2
    return out_sum / T

Strategy: data-parallel over batch (64 = 8 cores x 8). Per core:
  Phase A (time-parallel): XprojT[h, t*8+b] = scale*((x_t @ w1.T).T - LEAK)
      with x pre-transposed on host to xt[c, t*8+b]; fp16 hi/lo 3-pass
      (exact to ~fp32) or fp32r single-pass.
  Phase B (sequential scan over T): state transposed [H-chunkwise on
      partitions, B on free]: tiles [128, 32], free = 8*h_chunk + b.
      rec.T = w_rec @ spk.T via 16 matmuls (stationary = w_rec.T chunks,
      fp16 scaled by 64 to stay out of subnormals; v1 dynamics scaled
      by 64 too, numerically transparent). Spikes are written straight
      into spk_store (fp16) which feeds both the matmuls and phase C.
  Phase C (time-parallel): i2 = spk @ w2.T batched; the v2 relu
      accumulator is a first-order recurrence = one tensor_tensor_scan
      (state = max(state + d_t, 0)) over [O*B, T] layout; reduce-sum
      gives out_sum.

Modes (env SNN_MODE): "fp32" exact | "fp16" (default) | "f32r"
(fp32r phase A - only if probed exact on HW).
"""

import os
import numpy as np

B, T, C, H, O = 64, 256, 2048, 512, 11
THRESHOLD = 1.0
LEAK = 0.003
OUTPUT_LEAK = LEAK * 0.5

NCORES = 8
BL = B // NCORES          # batch per core = 8
BT = T * BL               # 2048 moving columns per core
P = 128
KC = C // P               # 16 contraction chunks for phase A
KH = H // P               # 4 H chunks
NTILE = 512               # phase A moving tile (64 timesteps x 8 batch)

MODE = os.environ.get("SNN_MODE", "fp16")

# fp16 mode: scale v1 dynamics so fp16(SCALE*w_rec) avoids subnormals.
SCALE = 1.0 if MODE == "fp32" else 64.0
LO_SHIFT = 2048.0  # 2^11: scaling for the w1 low-part in fp16 3-pass


def build_nc(t_steps=T, mode=MODE, debug_out=False):
    """Build the Bass program (same program for all 8 cores)."""
    from contextlib import ExitStack

    import concourse.tile as tile
    from concourse import bacc, mybir

    f32 = mybir.dt.float32
    f16 = mybir.dt.float16
    f32r = mybir.dt.float32r
    wdt = f32 if mode == "fp32" else f16
    alu = mybir.AluOpType
    ACT = mybir.ActivationFunctionType

    nc = bacc.Bacc("TRN2", target_bir_lowering=False, debug=False,
                   num_devices=NCORES)

    # ---- DRAM I/O ----
    if mode == "fp16":
        xh_d = nc.dram_tensor("xh", [C, BT], f16, kind="ExternalInput")
        xl_d = nc.dram_tensor("xl", [C, BT], f16, kind="ExternalInput")
        xs_d = nc.dram_tensor("xs", [C, BT], f16, kind="ExternalInput")
        w1h_d = nc.dram_tensor("w1h", [C, H], f16, kind="ExternalInput")
        w1l_d = nc.dram_tensor("w1l", [C, H], f16, kind="ExternalInput")
    else:
        adt = f32 if mode == "fp32" else f32r
        xt_d = nc.dram_tensor("xt", [C, BT], adt, kind="ExternalInput")
        w1t_d = nc.dram_tensor("w1t", [C, H], adt, kind="ExternalInput")
    wrt_d = nc.dram_tensor("wrt", [H, H], wdt, kind="ExternalInput")
    w2t_d = nc.dram_tensor("w2t", [H, O], wdt, kind="ExternalInput")
    out_d = nc.dram_tensor("out", [O, BL], f32, kind="ExternalOutput")
    if debug_out:
        v1_d = nc.dram_tensor("v1_dbg", [P, KH * BL], f32, kind="ExternalOutput")
        xp_d = nc.dram_tensor("xp_dbg", [P, 32 * t_steps], f32, kind="ExternalOutput")

    TH_S = SCALE * THRESHOLD
    spk_dt = wdt

    with tile.TileContext(nc) as tc, ExitStack() as ctx:
        # ---- persistent tiles (one pool, one slot per tag) ----
        perm = ctx.enter_context(tc.tile_pool(name="perm", bufs=1))

        def ptile(shape, dt_, tag):
            return perm.tile(shape, dt_, tag=tag, name=tag)

        xproj = ptile([P, 32 * t_steps], f32, "xproj")
        spk_store = ptile([P, 32 * t_steps], spk_dt, "spk_store")
        v1 = ptile([P, KH * BL], f32, "v1")

        w1_sb = []
        if mode == "fp16":
            for k in range(KC):
                w1h_k = ptile([P, H], f16, f"w1h_{k}")
                w1l_k = ptile([P, H], f16, f"w1l_{k}")
                nc.sync.dma_start(out=w1h_k[:], in_=w1h_d.ap()[k * P:(k + 1) * P, :])
                nc.sync.dma_start(out=w1l_k[:], in_=w1l_d.ap()[k * P:(k + 1) * P, :])
                w1_sb.append((w1h_k, w1l_k))
        else:
            for k in range(KC):
                w1_k = ptile([P, H], adt, f"w1_{k}")
                nc.sync.dma_start(out=w1_k[:], in_=w1t_d.ap()[k * P:(k + 1) * P, :])
                w1_sb.append(w1_k)

        wr_sb = []
        w2_sb = []
        for k in range(KH):
            wr_k = ptile([P, H], wdt, f"wr_{k}")
            nc.sync.dma_start(out=wr_k[:], in_=wrt_d.ap()[k * P:(k + 1) * P, :])
            wr_sb.append(wr_k)
            w2_k = ptile([P, O], wdt, f"w2_{k}")
            nc.sync.dma_start(out=w2_k[:], in_=w2t_d.ap()[k * P:(k + 1) * P, :])
            w2_sb.append(w2_k)

        nc.vector.memset(v1[:], 0.0)

        xt_pool = ctx.enter_context(tc.tile_pool(name="xt", bufs=4))
        psA_pool = ctx.enter_context(tc.tile_pool(name="psA", bufs=1, space="PSUM"))
        psB_pool = ctx.enter_context(tc.tile_pool(name="psB", bufs=3, space="PSUM"))

        # ================= Phase A: XprojT =================
        # xproj[p, 32t + 8j + b] = SCALE * ((x_t @ w1.T)[b, 128j+p] - LEAK)
        ntile = min(NTILE, t_steps * BL)
        n_steps_per_tile = ntile // BL
        nA = (t_steps * BL) // ntile
        for n in range(nA):
            psA = [psA_pool.tile([P, ntile], f32, tag=f"psA{j}", name=f"psA{j}")
                   for j in range(KH)]
            for k in range(KC):
                csl = slice(k * P, (k + 1) * P)
                nsl = slice(n * ntile, (n + 1) * ntile)
                if mode == "fp16":
                    xh_t = xt_pool.tile([P, ntile], f16, tag="xh", name="xh_t")
                    xl_t = xt_pool.tile([P, ntile], f16, tag="xl", name="xl_t")
                    xs_t = xt_pool.tile([P, ntile], f16, tag="xs", name="xs_t")
                    nc.sync.dma_start(out=xh_t[:], in_=xh_d.ap()[csl, nsl])
                    nc.sync.dma_start(out=xl_t[:], in_=xl_d.ap()[csl, nsl])
                    nc.sync.dma_start(out=xs_t[:], in_=xs_d.ap()[csl, nsl])
                    for j in range(KH):
                        st_h = w1_sb[k][0][:][:, j * P:(j + 1) * P]
                        st_l = w1_sb[k][1][:][:, j * P:(j + 1) * P]
                        nc.tensor.matmul(psA[j][:], st_h, xh_t[:],
                                         start=(k == 0), stop=False)
                        nc.tensor.matmul(psA[j][:], st_h, xl_t[:],
                                         start=False, stop=False)
                        nc.tensor.matmul(psA[j][:], st_l, xs_t[:],
                                         start=False, stop=(k == KC - 1))
                else:
                    xt_t = xt_pool.tile([P, ntile], adt, tag="xt", name="xt_t")
                    nc.sync.dma_start(out=xt_t[:], in_=xt_d.ap()[csl, nsl])
                    for j in range(KH):
                        st = w1_sb[k][:][:, j * P:(j + 1) * P]
                        nc.tensor.matmul(psA[j][:], st, xt_t[:],
                                         start=(k == 0), stop=(k == KC - 1))
            # copy PSUM -> xproj (strided dest), fusing scale and leak
            for j in range(KH):
                dest = xproj[:].rearrange("p (t j b) -> p t j b", j=KH, b=BL)[
                    :, n * n_steps_per_tile:(n + 1) * n_steps_per_tile, j, :]
                src = psA[j][:].rearrange("p (t b) -> p t b", b=BL)
                nc.scalar.activation(dest, src, ACT.Copy,
                                     bias=-SCALE * LEAK, scale=SCALE)

        # ================= Phase B: sequential scan =================
        psB_prev = None
        for t in range(t_steps):
            sl = slice(32 * t, 32 * t + 32)
            sl0 = slice(32 * t, 32 * t + 16)
            sl1 = slice(32 * t + 16, 32 * t + 32)
            # xp adds first: only depend on v1 post-reset(t-1); overlap PE(t-1)
            nc.vector.tensor_add(v1[:, 0:16], v1[:, 0:16], xproj[:, sl0])
            nc.vector.tensor_add(v1[:, 16:32], v1[:, 16:32], xproj[:, sl1])
            if psB_prev is not None:
                nc.vector.tensor_add(v1[:, 0:16], v1[:, 0:16], psB_prev[:, 0:16])
            nc.vector.tensor_scalar(spk_store[:, sl0], v1[:, 0:16], TH_S, None,
                                    alu.is_ge)
            if psB_prev is not None:
                nc.vector.tensor_add(v1[:, 16:32], v1[:, 16:32], psB_prev[:, 16:32])
            nc.vector.tensor_scalar(spk_store[:, sl1], v1[:, 16:32], TH_S, None,
                                    alu.is_ge)
            # subtractive reset: v1 = (-TH_S * spk) + v1
            nc.vector.scalar_tensor_tensor(v1[:], spk_store[:, sl], -TH_S, v1[:],
                                           alu.mult, alu.add)

            # ---- PE: rec for step t (feeds v1 update of step t+1) ----
            psB = psB_pool.tile([P, KH * BL], f32, tag="psB", name="psB")
            for j in range(KH):
                for k in range(KH):
                    nc.tensor.matmul(
                        psB[:, BL * j:BL * (j + 1)],
                        wr_sb[k][:][:, j * P:(j + 1) * P],
                        spk_store[:, 32 * t + BL * k:32 * t + BL * (k + 1)],
                        start=(k == 0), stop=(k == KH - 1))
            psB_prev = psB

        # ================= Phase C: v2 accumulator =================
        with tc.tile_pool(name="psV", bufs=1, space="PSUM") as psV_pool, \
             tc.tile_pool(name="phC", bufs=1) as phC_pool:
            d_all = phC_pool.tile([O, t_steps * BL], f32, tag="d_all", name="d_all")
            nsteps = ntile // BL
            spk_r = spk_store[:].rearrange("p (t c b) -> p t c b", c=KH, b=BL)
            for n in range(nA):
                psV = psV_pool.tile([O, ntile], f32, tag="psV", name="psV")
                for k in range(KH):
                    rhs = spk_r[:, n * nsteps:(n + 1) * nsteps, k, :]
                    nc.tensor.matmul(psV[:], w2_sb[k][:], rhs,
                                     start=(k == 0), stop=(k == KH - 1))
                # d = (i2 - OL) / T   (scan and sum are homogeneous in scale)
                nc.vector.tensor_scalar(
                    d_all[:, n * ntile:(n + 1) * ntile], psV[:],
                    -OUTPUT_LEAK, 1.0 / float(T), alu.add, alu.mult)
            # rearrange [O, (n t b)] -> [O*BL, t] via SBUF->SBUF DMA
            st2 = phC_pool.tile([O * BL, t_steps], f32, tag="st2", name="st2")
            zeros = phC_pool.tile([O * BL, t_steps], f32, tag="zeros", name="zeros")
            v2a = phC_pool.tile([O * BL, t_steps], f32, tag="v2a", name="v2a")
            osum88 = phC_pool.tile([O * BL, 1], f32, tag="osum88", name="osum88")
            nc.vector.memset(zeros[:], 0.0)
            # st2 row layout: partition = b*O + o (contiguous 11-row blocks)
            d_r = d_all[:].rearrange("o (n t b) -> o b (n t)", t=nsteps, b=BL)
            st2_r = st2[:].rearrange("(b o) t -> b o t", o=O)
            for b in range(BL):
                nc.sync.dma_start(out=st2_r[b, :, :], in_=d_r[:, b, :])
            # v2_t = max(v2_{t-1} + d_t, 0): one scan along t per (o,b) row
            nc.vector.tensor_tensor_scan(v2a[:], st2[:], zeros[:], 0.0,
                                         alu.add, alu.max)
            nc.vector.tensor_reduce(out=osum88[:], in_=v2a[:],
                                    axis=mybir.AxisListType.X, op=alu.add)
            nc.sync.dma_start(out=out_d.ap()[:, :].rearrange("o b -> b o"),
                              in_=osum88[:])

        if debug_out:
            nc.sync.dma_start(out=v1_d.ap()[:, :], in_=v1[:])
            nc.sync.dma_start(out=xp_d.ap()[:, :], in_=xproj[:])

    nc.compile()
    return nc


def prep_inputs(x, w1, w_rec, w2, mode=MODE):
    """Build per-core input maps. Host-side transposes/splits (not timed)."""
    x = np.ascontiguousarray(x, dtype=np.float32)
    w1 = np.ascontiguousarray(w1, dtype=np.float32)
    w_rec = np.ascontiguousarray(w_rec, dtype=np.float32)
    w2 = np.ascontiguousarray(w2, dtype=np.float32)

    if mode == "fp32":
        wrt = np.ascontiguousarray(w_rec.T) * np.float32(SCALE)
        w2t = np.ascontiguousarray(w2.T)
    else:
        wrt = (w_rec.T * SCALE).astype(np.float16)
        w2t = w2.T.astype(np.float16)

    in_maps = []
    if mode == "fp16":
        w1h = w1.T.astype(np.float16)                              # [C, H]
        w1l = ((w1.T - w1h.astype(np.float32)) * LO_SHIFT).astype(np.float16)
        for c in range(NCORES):
            xc = x[c * BL:(c + 1) * BL]                            # [BL, T, C]
            xt = np.ascontiguousarray(xc.transpose(2, 1, 0).reshape(C, BT))
            xh = xt.astype(np.float16)
            xl = (xt - xh.astype(np.float32)).astype(np.float16)
            xs = (xh.astype(np.float32) / LO_SHIFT).astype(np.float16)
            in_maps.append({"xh": xh, "xl": xl, "xs": xs,
                            "w1h": w1h, "w1l": w1l, "wrt": wrt, "w2t": w2t})
    else:
        w1t = np.ascontiguousarray(w1.T)
        for c in range(NCORES):
            xc = x[c * BL:(c + 1) * BL]
            xt = np.ascontiguousarray(xc.transpose(2, 1, 0).reshape(C, BT))
            in_maps.append({"xt": xt, "w1t": w1t, "wrt": wrt, "w2t": w2t})
    return in_maps


_LAST = {"exec_time_ns": None, "results": None}


def kernel(x, w1, w_rec, w2):
    from concourse.bass_utils import run_bass_kernel_spmd

    nc = build_nc()
    in_maps = prep_inputs(x, w1, w_rec, w2)
    trace = os.environ.get("SNN_TRACE", "0") == "1"
    if trace:
        try:
            import antenv
            if "/opt/trn_rl_repo/antenv" not in antenv.__path__:
                antenv.__path__.append("/opt/trn_rl_repo/antenv")
            import antenv.axon_hooks  # noqa: F401
        except Exception:
            trace = False
    res = run_bass_kernel_spmd(nc, in_maps, list(range(NCORES)), trace=trace)
    _LAST["exec_time_ns"] = res.exec_time_ns
    _LAST["results"] = res
    out = np.empty((B, O), dtype=np.float32)
    for c in range(NCORES):
        out[c * BL:(c + 1) * BL, :] = res.results[c]["out"].T
    return out
